# revision 51
# baseline (speedup 1.0000x reference)
"""BMC loss (InfoNCE-style MVN loss) on 8 trn2 NeuronCores.

loss = mean_i( LSE_j(u_ij/nv) - u_ii/nv ) * 2*nv,  u_ij = p_i.t_j - 0.5||t_j||^2
(the ||p_i||^2 and log-norm terms cancel between the logit and its row LSE)

Sharding: pred rows split across 8 cores (slab=1024 rows each), target
replicated.  Host does all O(B) / O(B*D) work (t2, diag, final ln/mean);
the device computes s_i = sum_j exp((u_ij + S)/nv) with S = -max_i u_ii.

v2 architecture (dual-engine exp):  all 64 j-chunks per core use the
transposed layout [j on partitions, i on free].  Per chunk: fp8 DoubleRow
cross-matmul -> PSUM f32 logits -> exp -> bf16 E tile -> ones-stationary
matmul accumulates partition sums into a persistent PSUM accumulator
(PSUM: 3 double-buffered [128,1024] logit tiles + the accumulator = all
8 banks).  The exp alternates strictly a,d,a,d across TWO engines,
breaking v1's single-engine exp floor (ACT busy 65.5us):

- ACT chunks (32): hardware Exp, per-partition bias (S - t2_j)/nv.
- DVE chunks (32): Schraudolph bit-trick exp in ONE tensor_scalar op:
  bits = round((c + s1_j) * 184.6627/nv) -> uint16 (the f32->uint16
  writeback rounds-to-nearest and saturates, so junk tails clamp to
  +0.0 bf16), bitcast to bf16 = 2^(bits/128 - 127) ~= e^l with ~±4%
  mantissa-interp noise, zero-mean after the magic-constant calibration
  (K = 16256 - 7.37).  Measured loss error is unchanged vs v1 (3.5e-4,
  dominated by the shared fp8 input quantization; 57x inside the gate).

Schedule: ones-matmuls trail E production by ones_delay chunks, paced one
half per chunk between cross-matmuls (pend queue sorted by readiness);
6 dummy matmuls on a memset tile hold the PE p-state through its 3us
ramp (the first real ones-matmul start=True reset discards them); the
last chunk's exp is split so its ones-halves chase it, the ones matrix
is built by an on-device memset (one fewer serialized HWDGE issue), and
the final accumulator row is evacuated split across DVE+ACT before one
output DMA (two split DMAs lose ~0.7us to the serialized issue+sem path).

Cost-model steady state: DVE-bound at 1192+88ns per a,d pair (640/chunk);
engine busy PE 43.7 (incl 2.9 warm) / DVE 39.4 / ACT 35.3us.  Merging
E-pairs (Pool or DVE) to relieve PE always lost more to pipeline jitter
than it saved.  Packing pred+target column-wise into one dram tensor
lets a single startup DMA (625ns HWDGE issue + 900ns sem each) deliver
all of pred plus the first target chunks: -1.0us off the lead-in.
TimelineSim: 50,043 ns vs 76,996 ns for v1 (1.54x).
"""

import numpy as np

B = 8192
D = 256
NCORES = 8
P = 128
SLAB = B // NCORES          # pred rows per core
KC = D // P                 # contraction chunks
NCH = B // P                # j-chunks per core (64)
JT = 512                    # matmul moving free dim (one PSUM bank)

# Schraudolph constants (bf16 bit trick): bits = l*SCHRAUD_A + SCHRAUD_K
SCHRAUD_A = 128.0 / float(np.log(2.0))        # 184.6627
SCHRAUD_K = 16256.0 - 7.37                    # 127*128 - mean-error calib

# tunables (must match between _build and the host-side kernel())
N_DVE = 32                  # chunks exp'd on DVE (rest on ACT)
POOL_PAIRS = 0              # E-tile pairs pre-merged on GPSIMD (hurts: serial hop)
ONES_DELAY = 4              # chunks between E production and its ones-matmul


def _layout(n_dve=N_DVE, dve_pairs=0, pool_pairs=POOL_PAIRS, nch=NCH):
    """Build (path, merge_ks) in PAIR units so merged pairs are homogeneous
    'dd' (the merge then only depends on the DVE engine's own outputs — no
    cross-engine head-of-line stall).  Mixed pairs are 'da'; the final pair
    is mixed so both engines run to the end and the last chunk is ACT."""
    npairs = nch // 2
    m = dve_pairs + pool_pairs
    nmix = n_dve - 2 * m           # pairs with a single 'd'
    assert nmix >= 0, "n_dve too small for the merge-pair count"
    naa = npairs - m - nmix
    assert naa >= 0, "n_dve too large for the merge-pair count"
    # interleave pair types evenly (largest remainder), reserving the final
    # pair for a mixed 'da' (or 'aa' if no mixed pairs remain)
    counts = {'dd': m, 'da': nmix, 'aa': naa}
    last = 'da' if counts['da'] > 0 else 'aa'
    counts[last] -= 1
    seq = []
    acc = {k: 0.0 for k in counts}
    tot = max(npairs - 1, 1)
    for i in range(npairs - 1):
        for k in counts:
            acc[k] += counts[k] / tot
        pick = max(acc, key=lambda k: acc[k])
        if acc[pick] <= 0:
            pick = next(k for k in counts if sum(1 for s in seq if s == k)
                        < counts[k])
        # choose the type furthest behind its quota
        done = {k: sum(1 for s in seq if s == k) for k in counts}
        pick = max(counts, key=lambda k: counts[k] * (i + 1) / tot - done[k])
        seq.append(pick)
    seq.append(last)
    path = []
    mks = {}
    merge_engines = ['dve'] * dve_pairs + ['pool'] * pool_pairs
    mi = 0
    for k, typ in enumerate(seq):
        if typ == 'dd':
            path += ['d', 'd']
            mks[k] = merge_engines[mi % max(len(merge_engines), 1)]
            mi += 1
        elif typ == 'da':
            path += ['d', 'a']
        else:
            path += ['a', 'a']
    assert path.count('d') == n_dve and len(path) == nch
    assert path[nch - 1] == 'a'
    return path, mks


def _build(n_dve=N_DVE, pool_pairs=POOL_PAIRS, dve_pairs=0,
           ones_delay=ONES_DELAY,
           tgt_blocks=(4, 12, 16, 16, 16), ebufs=10, mbufs=6, tpbufs=3,
           evac="copy", warm_mms=6, dve_split=0, pool_sp=0, pool_lat=4,
           dve_sp=0):
    import concourse.bass as bass
    import concourse.mybir as mybir
    import concourse.tile as tile
    from concourse import bacc
    from contextlib import ExitStack

    f32 = mybir.dt.float32
    bf16 = mybir.dt.bfloat16
    u16 = mybir.dt.uint16
    f8 = mybir.dt.float8e4

    path, mks = _layout(n_dve, dve_pairs, pool_pairs)
    assert sum(tgt_blocks) == NCH
    # spaced same-engine pool merges: pair consecutive chunks OF THE SAME
    # exp engine (2 apart in the adad layout) so the merge never waits on
    # the other engine and the alternation stays intact
    # DVE self-merges parked in the 'aa' doublet slots: when ACT runs two
    # consecutive chunks, DVE is idle ~1.3us — merge its two most recent
    # E-tiles there (inputs are DVE's own completed exps, zero wait) and
    # save the pair's second ones-matmul on the PE
    dve_sp_at = {}                 # first-a-chunk -> (d1, d2) to merge
    if dve_sp:
        held = set()
        recent = []
        n_used = 0
        for c, pch in enumerate(path[:-3]):
            if pch == 'd':
                recent.append(c)
            elif (n_used < dve_sp and c + 1 < len(path) - 2
                  and path[c + 1] == 'a' and len(recent) >= 2
                  and recent[-1] == c - 1):
                d2 = recent.pop(); d1 = recent.pop()
                dve_sp_at[c] = (d1, d2)
                held.add(d1); held.add(d2)
                n_used += 1
    sp_partner = {}
    if pool_sp:
        for ch in ('d', 'a'):
            idxs = [i for i, p in enumerate(path) if p == ch][:-2]
            prs = [(idxs[2 * k], idxs[2 * k + 1])
                   for k in range(len(idxs) // 2)]
            take = min(pool_sp, len(prs))
            step = len(prs) / max(take, 1)
            for k in range(take):
                x, y = prs[min(int(k * step), len(prs) - 1)]
                if x not in sp_partner and y not in sp_partner:
                    sp_partner[y] = x

    W = SLAB + B
    nc = bacc.Bacc("TRN2", target_bir_lowering=False, debug=False)
    # pred and target packed column-wise in ONE dram tensor: the first DMA
    # delivers all of pred + the first target chunks in a single serialized
    # HWDGE issue (each issue costs 625ns + a 900ns completion sem)
    pt = nc.dram_tensor("pt", [D, W], f8, kind="ExternalInput")
    # smalls cols: [0:NCH] act bias (S-t2_j)/nv; [NCH:2*NCH] dve schraudolph
    # bias s1_j; [2*NCH] 1/nv; [2*NCH+1] SCHRAUD_A/nv
    smalls = nc.dram_tensor("smalls", [P, 2 * NCH + 2], f32, kind="ExternalInput")
    s_out = nc.dram_tensor("s_out", [1, SLAB], f32, kind="ExternalOutput")

    with ExitStack() as ctx:
        tc = ctx.enter_context(tile.TileContext(nc))
        singles = ctx.enter_context(tc.tile_pool(name="singles", bufs=1))
        tpool = ctx.enter_context(tc.tile_pool(name="tpool", bufs=tpbufs,
                                               space="PSUM"))
        apool = ctx.enter_context(tc.tile_pool(name="apool", bufs=1,
                                               space="PSUM"))
        epool = ctx.enter_context(tc.tile_pool(name="epool", bufs=ebufs))
        mpool = ctx.enter_context(tc.tile_pool(name="mpool", bufs=mbufs))

        pt_sb = singles.tile([P, KC, W], f8)
        predT_sb = pt_sb[:, :, 0:SLAB]
        targetT_sb = pt_sb[:, :, SLAB : SLAB + B]
        smalls_sb = singles.tile([P, 2 * NCH + 2], f32)
        ones_sb = singles.tile([P, P], bf16)
        warm = singles.tile([P, 1], f32)
        invnv_sb = smalls_sb[:, 2 * NCH : 2 * NCH + 1]
        schrA_sb = smalls_sb[:, 2 * NCH + 1 : 2 * NCH + 2]

        # PE p-state warm-up: dummy matmuls on a memset tile keep the PE
        # continuously busy through its 3us ramp window while input DMAs
        # stream, so every real matmul runs at the full 2.4GHz clock.  The
        # first real ones-matmul resets the accumulator bank (start=True),
        # discarding the dummy results.  (wsrc memset first: it gates PE.)
        if warm_mms:
            wsrc = singles.tile([P, JT], bf16)
            nc.vector.memset(wsrc, 0.0)
        nc.vector.memset(ones_sb, 1.0)

        # preload the exp table set at t~0 (real-HW nicety; TimelineSim
        # charges no table loads)
        nc.vector.memset(warm, 0.0)
        nc.scalar.activation(out=warm, in_=warm,
                             func=mybir.ActivationFunctionType.Exp)

        # ---- input DMAs in consumption order (single HWDGE + serialized
        # transfer pipe: order == availability) ----
        def load_pt(lo, hi):
            nc.sync.dma_start(
                out=pt_sb[:, :, lo:hi],
                in_=bass.AP(tensor=pt[0:P, lo:hi].tensor, offset=lo,
                            ap=[[W, P], [P * W, KC], [1, hi - lo]]),
            )

        head = tgt_blocks[0]
        load_pt(0, SLAB + head * P)      # all of pred + first tgt chunks
        nc.sync.dma_start(out=smalls_sb, in_=smalls[:, :])
        lo = SLAB + head * P
        for blk in tgt_blocks[1:]:
            hi = lo + blk * P
            load_pt(lo, hi)
            lo = hi
        assert lo == W

        acc = apool.tile([P, SLAB], f32, tag="acc")
        for _w in range(warm_mms):
            nc.tensor.matmul(
                out=acc[:, 0:JT], lhsT=wsrc[:, 0:P], rhs=wsrc,
                start=True, stop=True,
            )
        e_tiles = {}       # chunk -> E tile (bf16 view)
        n_units_total = NCH - len(mks) - len(sp_partner) - len(dve_sp_at)
        NH = SLAB // JT    # i-halves per unit
        emitted_h = [0, 0]       # ones-halves emitted per bank
        pend = []                # (rhs AP, h, ready_at_chunk) FIFO

        def emit_half():
            rhs, h, _ = pend.pop(0)
            first = emitted_h[h] == 0
            last = emitted_h[h] == n_units_total - 1
            nc.tensor.matmul(
                out=acc[:, h * JT : (h + 1) * JT],
                lhsT=ones_sb,
                rhs=rhs[:, h * JT : (h + 1) * JT],
                start=first,
                stop=last,
            )
            emitted_h[h] += 1

        def pump(now, cap):
            # emit up to cap pending ones-halves whose unit is >= ones_delay
            # chunks old
            n = 0
            while pend and n < cap and pend[0][2] <= now - ones_delay:
                emit_half()
                n += 1

        def queue_unit(rhs, at):
            for h in range(NH):
                pend.append((rhs, h, at))
            pend.sort(key=lambda x: x[2])

        def emit_exp(e, tp, c, lo, w, eng=None):
            if (eng or path[c]) == 'a':
                nc.scalar.activation(
                    out=e[:, lo : lo + w], in_=tp[:, lo : lo + w],
                    func=mybir.ActivationFunctionType.Exp,
                    bias=smalls_sb[:, c : c + 1],
                    scale=invnv_sb,
                )
            else:
                nc.vector.tensor_scalar(
                    out=e.bitcast(u16)[:, lo : lo + w], in0=tp[:, lo : lo + w],
                    scalar1=smalls_sb[:, NCH + c : NCH + c + 1],
                    scalar2=schrA_sb,
                    op0=mybir.AluOpType.add,
                    op1=mybir.AluOpType.mult,
                )

        def emit_exp_maybe_split(e, tp, c):
            if path[c] == 'd' and dve_split:
                emit_exp(e, tp, c, 0, JT)
                emit_exp(e, tp, c, JT, SLAB - JT)
            else:
                emit_exp(e, tp, c, 0, SLAB)

        for c in range(NCH):
            # cross matmuls for chunk c back-to-back (the exp needs BOTH
            # halves; a ones-half between them would delay tp by 213ns),
            # then drain pending ones-halves
            tp = tpool.tile([P, SLAB], f32, tag="tp")
            for h in range(NH):
                nc.tensor.matmul(
                    out=tp[:, h * JT : (h + 1) * JT],
                    lhsT=targetT_sb[:, :, c * P : (c + 1) * P],
                    rhs=predT_sb[:, :, h * JT : (h + 1) * JT],
                    start=True,
                    stop=True,
                    perf_mode=mybir.MatmulPerfMode.DoubleRow,
                )
            pump(c, 2)

            e = epool.tile([P, SLAB], bf16, tag="e")
            if c == NCH - 1:
                # tail: split the last exp and chase it with its ones-halves
                assert path[c] == 'a' and (c // 2) not in mks
                for cc in sorted(e_tiles):
                    queue_unit(e_tiles.pop(cc), cc)
                while pend:        # all earlier units precede the stop flags
                    emit_half()
                for h in range(NH):
                    emit_exp(e, tp, c, h * JT, JT)
                    pend.append((e, h, c))
                    emit_half()
                continue

            emit_exp_maybe_split(e, tp, c)
            e_tiles[c] = e

            # merge or queue the finished chunks
            if c in dve_sp_at:
                d1, d2 = dve_sp_at[c]
                m = mpool.tile([P, SLAB], bf16, tag="m")
                nc.vector.tensor_tensor(
                    m, e_tiles.pop(d1), e_tiles.pop(d2),
                    mybir.AluOpType.add,
                )
                queue_unit(m, c + 1)
            if c in sp_partner:
                m = mpool.tile([P, SLAB], bf16, tag="m")
                nc.gpsimd.tensor_tensor(
                    m, e_tiles.pop(sp_partner[c]), e_tiles.pop(c),
                    mybir.AluOpType.add,
                )
                queue_unit(m, c + pool_lat)
            elif c % 2 == 1 and (c // 2) in mks:
                k = c // 2
                eng = nc.gpsimd if mks[k] == 'pool' else nc.vector
                lat = 2 if mks[k] == 'pool' else 1
                m = mpool.tile([P, SLAB], bf16, tag="m")
                eng.tensor_tensor(
                    m, e_tiles.pop(c - 1), e_tiles.pop(c),
                    mybir.AluOpType.add,
                )
                queue_unit(m, c + lat)
            else:
                later = set(sp_partner) | set(sp_partner.values())
                for _d1, _d2 in dve_sp_at.values():
                    later.add(_d1); later.add(_d2)
                for cc in sorted(e_tiles):
                    if cc not in later:
                        queue_unit(e_tiles.pop(cc), cc)
            pump(c, 1)

        assert emitted_h == [n_units_total] * NH, emitted_h

        # evacuate row 0 of the accumulator (split across DVE and ACT so the
        # two halves run in parallel on the tail)
        s_row = singles.tile([1, SLAB], f32)
        nc.vector.tensor_copy(s_row[:, 0:JT], acc[0:1, 0:JT])
        nc.scalar.activation(out=s_row[:, JT:SLAB], in_=acc[0:1, JT:SLAB],
                             func=mybir.ActivationFunctionType.Copy)
        nc.sync.dma_start(out=s_out[:, :], in_=s_row)

    nc.compile()
    return nc


_NC = None
_TRACE = False
_LAST_RESULT = [None]
_ONES_BF = None


def kernel(pred, target, noise_sigma):
    global _NC, _ONES_BF
    import ml_dtypes
    from concourse.bass_utils import run_bass_kernel_spmd

    pred = np.ascontiguousarray(np.asarray(pred, dtype=np.float32))
    target = np.ascontiguousarray(np.asarray(target, dtype=np.float32))
    nv = float(np.asarray(noise_sigma, dtype=np.float64) ** 2)

    if _NC is None:
        _NC = _build()
    if _ONES_BF is None:
        _ONES_BF = np.ones((P, P), dtype=ml_dtypes.bfloat16)

    t64 = target.astype(np.float64)
    p64 = pred.astype(np.float64)
    t2 = 0.5 * (t64 * t64).sum(axis=1)              # [B]
    diag = np.einsum("ij,ij->i", p64, t64)          # [B]
    u_ii = diag - t2
    S = float(-np.max(u_ii))

    smalls = np.zeros((P, 2 * NCH + 2), dtype=np.float32)
    bias = ((S - t2) / nv).reshape(NCH, P).T        # [P, NCH]
    smalls[:, :NCH] = bias
    # dve: bits = (c + s1_j) * (SCHRAUD_A/nv); s1 = (S - t2_j) + K*nv/A
    smalls[:, NCH : 2 * NCH] = (bias * nv) + SCHRAUD_K * nv / SCHRAUD_A
    smalls[:, 2 * NCH] = 1.0 / nv
    smalls[:, 2 * NCH + 1] = SCHRAUD_A / nv

    predT_b = pred.T.astype(ml_dtypes.float8_e4m3fn)
    targetT_b = target.T.astype(ml_dtypes.float8_e4m3fn)
    in_maps = []
    for c in range(NCORES):
        in_maps.append(
            {
                "pt": np.ascontiguousarray(np.concatenate(
                    [predT_b[:, c * SLAB : (c + 1) * SLAB], targetT_b],
                    axis=1)),
                "smalls": smalls,
            }
        )

    kw = {}
    if _TRACE:
        kw = dict(trace=True, stitch_traces=False)
    res = run_bass_kernel_spmd(_NC, in_maps, core_ids=list(range(NCORES)), **kw)
    _LAST_RESULT[0] = res

    s_tot = np.zeros(B, dtype=np.float64)
    for c, r in enumerate(res.results):
        s_tot[c * SLAB : (c + 1) * SLAB] = r["s_out"].astype(np.float64)[0]

    lse = np.log(s_tot) - S / nv
    loss = 2.0 * nv * np.mean(lse - u_ii / nv)
    return np.asarray(loss, dtype=np.float32)
